# revision 1
# baseline (speedup 1.0000x reference)
"""GAT (3 layers, heads=1) + global-max-pool + MLP head on 8 Trainium2 NeuronCores.

Sharding: 64 graphs -> 8 cores (8 graphs each; batch is sorted so graphs are
contiguous node ranges).  Graph slot j on every core is padded to a common
length GL[j] so all cores run one identical NEFF (SPMD).  Within a graph,
nodes are sorted by descending in-degree (tightens padded-CSR slot grids).

Per layer: each core computes h_ext = [h | h@a_src] rows (bf16, 256B) for its
own nodes, AllGathers the table to every core, then for each work item
(group of 128-node tiles) gathers the neighbor rows of its own edges with
dma_gather.  Indices are int16, so the table is addressed in 32768-row
windows: each node's neighbor list is split by window, and the slot grid has
per-(item, window) padded depth D_c.  Masked stable segment softmax and the
weighted feature sum run on DVE/ACT; the PE transposes each output tile and
applies the next layer's [W | W@a_src | W@a_dst] in a fused tail.

Performance state (2026-08-05): real problem rel-err 3.0e-4; wall 98.9 ms/call
of which ~74 ms is the axon PJRT dispatch floor -> device time ~25 ms.
TimelineSim (cost model, single-core, GAT_NOCC=1) predicts only 2.9 ms, so
~22 ms is unmodeled: prime suspects are the 3 AllGathers (27 MB each, absent
from the model run), SWDGE descriptor-ring writes for ~420k gather
descriptors/layer, and per-packet SDMA overhead with single_packet=False.
Measured via isolated probes (probe_ag.py / probe_gather.py, device-resident
timing minus the 74 ms floor):
- AllGather 27 MB x8 cores: ~1.95 ms each -> the 3 AGs = ~5.8 ms of the 25.
- dma_gather, single_packet=False, 4096-idx calls: ~50 ns/row (~21.5 ms for
  one layer's 426k rows in isolation) vs the 1.4 ns/row cost model -- the
  gather path is the dominant bottleneck (~16 ms of the 25 after overlap).
- single_packet=True with 512-idx calls (33 descs/engine, within the 64/pkt
  limit) should restore the fast path BUT 832 back-to-back calls crash the
  device: the SWDGE ring (16384 descs, ~1024/engine-lane) overflows without
  flow control.  Fix: bound outstanding gathers to <= ~25 calls (ring/33)
  via consuming ops or explicit sem waits, then re-measure.
Next steps, in order of expected win: (1) packetized 512-idx gathers with
flow control (potential ~16 -> ~4 ms); (2) overlap/chunk the AllGathers
(~5.8 ms, can hide under phase-B tails); (3) per-window degree-sorted grids
+ dma_scatter_add combine to cut the ~2.5x slot padding.
"""

import os
import sys
import numpy as np

DBG = int(os.environ.get("GAT_DBG", "0"))
MAXITEMS = int(os.environ.get("GAT_MAXITEMS", "9999"))
MAXCH = int(os.environ.get("GAT_MAXCH", "9999"))
NOPRO = int(os.environ.get("GAT_NOPRO", "0"))
NOCC = int(os.environ.get("GAT_NOCC", "0"))

for _p in ("/opt/trn_rl_repo", "/opt/trn_rl_repo/concourse"):
    if _p not in sys.path:
        sys.path.insert(0, _p)

import concourse.bass as bass  # noqa: E402
import concourse.bacc as bacc  # noqa: E402
import concourse.mybir as mybir  # noqa: E402
import concourse.tile as tile  # noqa: E402
from concourse import library_config  # noqa: E402
from concourse.masks import make_identity  # noqa: E402
from concourse.bass_utils import run_bass_kernel_spmd  # noqa: E402

F32 = mybir.dt.float32
BF16 = mybir.dt.bfloat16
I16 = mybir.dt.int16
ALU = mybir.AluOpType
ACTF = mybir.ActivationFunctionType
AX = mybir.AxisListType

NCORES = 8
NGRAPH = 64
CHUNK = 32768          # int16 index reach per dma_gather call
ROW = 128              # bf16 values per h_ext row = 256B
SLOT_BUDGET = 16384    # max gathered slots per work item (SBUF bound)
MAX_TILES = 16
NEG = -1.0e30
GMAX = 4096         # max idxs per dma_gather call (desc-ring bound)


def _ap(t, off, dims):
    return bass.AP(t, off, dims)


# ----------------------------------------------------------------------------
# Host-side preprocessing (sharding / layout + static CSR tables)
# ----------------------------------------------------------------------------

def _preprocess(adj, batch):
    N = batch.shape[0]
    gper = NGRAPH // NCORES
    graph_of = batch.astype(np.int64)
    counts = np.bincount(graph_of, minlength=NGRAPH)
    gstarts = np.zeros(NGRAPH + 1, np.int64)
    np.cumsum(counts, out=gstarts[1:])

    src = np.concatenate([adj[0].astype(np.int64), np.arange(N, dtype=np.int64)])
    dst = np.concatenate([adj[1].astype(np.int64), np.arange(N, dtype=np.int64)])
    deg = np.bincount(dst, minlength=N)

    # common padded per-graph-slot lengths
    glens = counts.reshape(NCORES, gper)
    GL = np.maximum(glens.max(axis=0), 1)            # [gper]
    GST = np.zeros(gper + 1, np.int64)
    np.cumsum(GL, out=GST[1:])
    NPADC = int(np.ceil(GST[-1] / 128) * 128)
    NT = NPADC // 128

    # per-graph degree-desc order; old -> new id (new = core*NPADC + col)
    order = np.lexsort((-deg, graph_of))             # old ids, grouped by graph
    new_of_old = np.empty(N, np.int64)
    order_padded = np.full((NCORES, NPADC), -1, np.int64)
    for g in range(NGRAPH):
        c, j = g // gper, g % gper
        olds = order[gstarts[g]:gstarts[g + 1]]
        col0 = GST[j]
        order_padded[c, col0:col0 + len(olds)] = olds
        new_of_old[olds] = c * NPADC + col0 + np.arange(len(olds))

    NTOT = NCORES * NPADC
    NCH = int((NTOT + CHUNK - 1) // CHUNK)

    nsrc = new_of_old[src]
    ndst = new_of_old[dst]
    dst_core = ndst // NPADC
    dst_local = ndst % NPADC
    ch_src = nsrc // CHUNK
    loc_src = (nsrc % CHUNK).astype(np.int32)

    # per-(core, local node, chunk) degree; max over cores
    degc = np.zeros((NCORES, NPADC, NCH), np.int32)
    np.add.at(degc, (dst_core, dst_local, ch_src), 1)
    degc_max = degc.max(axis=0)

    # shared work-item schedule
    tile_dc = degc_max.reshape(NT, 128, NCH).max(axis=1)
    items = []
    t0 = 0
    while t0 < NT:
        T = 1
        dcur = np.maximum(tile_dc[t0], 1)
        while t0 + T < NT and T < MAX_TILES:
            nd = np.maximum(np.maximum(dcur, tile_dc[t0 + T]), 1)
            if (T + 1) * 128 * int(nd.sum()) > SLOT_BUDGET:
                break
            dcur = nd
            T += 1
        items.append((t0, T, dcur.copy()))
        t0 += T

    # table layouts
    idx_cols = []   # per item: per chunk (col_off, ncols, num_idx)
    msk_cols = []   # per item: col_off
    icol = mcol = 0
    for (ts, T, dc) in items:
        S = int(dc.sum())
        msk_cols.append(mcol)
        mcol += T * S
        row = []
        for c in range(NCH):
            ni = 128 * T * int(dc[c])
            row.append((icol, ni // 16, ni))
            icol += ni // 16
        idx_cols.append(row)
    IDXCOLS, MSKCOLS = icol, mcol

    # per-node lookup arrays for vectorized fill
    item_of_tile = np.zeros(NT, np.int64)
    for ii, (ts, T, dc) in enumerate(items):
        item_of_tile[ts:ts + T] = ii
    arr_ts = np.array([it[0] for it in items], np.int64)
    arr_T = np.array([it[1] for it in items], np.int64)
    arr_S = np.array([int(it[2].sum()) for it in items], np.int64)
    arr_offd = np.zeros((len(items), NCH), np.int64)
    arr_cbase = np.zeros((len(items), NCH), np.int64)
    for ii in range(len(items)):
        off = 0
        for c in range(NCH):
            arr_offd[ii, c] = off
            arr_cbase[ii, c] = idx_cols[ii][c][0]
            off += int(items[ii][2][c])
    arr_mbase = np.array(msk_cols, np.int64)

    idx_tabs, msk_tabs, vlds = [], [], []
    for c in range(NCORES):
        m = dst_core == c
        o = np.lexsort((loc_src[m], ch_src[m], dst_local[m]))
        dl = dst_local[m][o]
        ch = ch_src[m][o]
        lo = loc_src[m][o]
        ne = len(dl)
        # rank within (node, chunk)
        if ne:
            keys = dl * NCH + ch
            brk = np.ones(ne, bool)
            brk[1:] = keys[1:] != keys[:-1]
            gid = np.cumsum(brk) - 1
            gst = np.zeros(gid[-1] + 2 if ne else 1, np.int64)
            np.add.at(gst[1:], gid, 1)
            np.cumsum(gst, out=gst)
            rank = np.arange(ne) - gst[gid]
        else:
            rank = np.zeros(0, np.int64)
        til = dl // 128
        p = dl % 128
        ii = item_of_tile[til]
        t = til - arr_ts[ii]
        T = arr_T[ii]
        S = arr_S[ii]
        D = items[0][2]  # placeholder
        # gather idx table
        q = rank * (T * 128) + t * 128 + p
        col = arr_cbase[ii, ch] + q // 16
        rrow = q % 16
        it = np.zeros((16, IDXCOLS), np.int16)
        it[rrow, col] = lo.astype(np.int16)
        idx_tabs.append(np.tile(it, (8, 1)))
        # mask table
        mt = np.full((128, MSKCOLS), NEG, np.float32)
        mcolv = arr_mbase[ii] + t * S + arr_offd[ii, ch] + rank
        mt[p, mcolv] = 0.0
        # pad nodes: unmask slot (chunk0, j=0) so den=1
        vld = np.zeros((128, NT), np.float32)
        padm = order_padded[c] < 0
        for ti in range(NT):
            iii = item_of_tile[ti]
            tt = ti - arr_ts[iii]
            SS = arr_S[iii]
            prow = np.nonzero(padm[ti * 128:(ti + 1) * 128])[0]
            mt[prow, arr_mbase[iii] + tt * SS] = 0.0
            vld[:, ti] = (~padm[ti * 128:(ti + 1) * 128]).astype(np.float32)
        msk_tabs.append(mt)
        vlds.append(vld)

    return dict(
        N=N, gper=gper, NPADC=NPADC, NT=NT, NTOT=NTOT, NCH=NCH,
        order_padded=order_padded, items=items,
        idx_cols=idx_cols, msk_cols=msk_cols,
        IDXCOLS=IDXCOLS, MSKCOLS=MSKCOLS,
        idx_tabs=idx_tabs, msk_tabs=msk_tabs, vlds=vlds,
        GL=[int(v) for v in GL], GST=[int(v) for v in GST],
        roots=gstarts[:NGRAPH].copy(),
    )


# ----------------------------------------------------------------------------
# Device program
# ----------------------------------------------------------------------------

def _build_program(pp, IN, HID):
    NPADC, NT, NTOT, NCH = pp["NPADC"], pp["NT"], pp["NTOT"], pp["NCH"]
    items, idx_cols, msk_cols = pp["items"], pp["idx_cols"], pp["msk_cols"]
    IDXCOLS, MSKCOLS = pp["IDXCOLS"], pp["MSKCOLS"]
    GL, GST, gper = pp["GL"], pp["GST"], pp["gper"]
    GLMAX = int(np.ceil(max(GL) / 128) * 128)

    nc = bacc.Bacc("TRN2", target_bir_lowering=False, debug=False,
                   num_devices=(1 if NOCC else NCORES))

    xT = nc.dram_tensor("xT", [IN, NPADC], F32, kind="ExternalInput")
    xrootT = nc.dram_tensor("xrootT", [IN, gper], F32, kind="ExternalInput")
    idx_t = nc.dram_tensor("idx", [128, IDXCOLS], I16, kind="ExternalInput")
    msk_t = nc.dram_tensor("msk", [128, MSKCOLS], F32, kind="ExternalInput")
    vld_t = nc.dram_tensor("vld", [128, NT], F32, kind="ExternalInput")
    Ws = {}
    for l, di in ((1, IN), (2, HID), (3, HID)):
        Ws[f"W{l}"] = nc.dram_tensor(f"W{l}", [di, HID], F32, kind="ExternalInput")
        Ws[f"as{l}"] = nc.dram_tensor(f"as{l}", [HID, 1], F32, kind="ExternalInput")
        Ws[f"ad{l}"] = nc.dram_tensor(f"ad{l}", [HID, 1], F32, kind="ExternalInput")
        Ws[f"b{l}"] = nc.dram_tensor(f"b{l}", [128, HID], F32, kind="ExternalInput")
    lin0W = nc.dram_tensor("lin0W", [HID, HID], F32, kind="ExternalInput")
    lin0b = nc.dram_tensor("lin0b", [gper, HID], F32, kind="ExternalInput")
    linnW = nc.dram_tensor("linnW", [IN, HID], F32, kind="ExternalInput")
    linnb = nc.dram_tensor("linnb", [gper, HID], F32, kind="ExternalInput")
    lin1W = nc.dram_tensor("lin1W", [2 * HID, 1], F32, kind="ExternalInput")
    lin1b = nc.dram_tensor("lin1b", [gper, 1], F32, kind="ExternalInput")
    ident_in = nc.dram_tensor("ident", [128, 128], F32, kind="ExternalInput")
    out_t = nc.dram_tensor("out", [gper, 1], F32, kind="ExternalOutput")

    agin = [nc.dram_tensor(f"agin{l}", [NPADC, ROW], BF16, kind="Internal")
            for l in range(3)]
    htab = [nc.dram_tensor(f"htab{l}", [NTOT, ROW], BF16, kind="Internal")
            for l in range(3)]
    x4T_d = nc.dram_tensor("x4T", [HID, NPADC], F32, kind="Internal")

    with tile.TileContext(nc) as tc:
        with (
            tc.tile_pool(name="const", bufs=1) as cpool,
            tc.tile_pool(name="gbuf", bufs=2) as gpool,
            tc.tile_pool(name="pbuf", bufs=2) as ppool,
            tc.tile_pool(name="sbuf", bufs=3) as spool,
            tc.tile_pool(name="psum", bufs=2, space="PSUM") as pspool,
            tc.tile_pool(name="psA", bufs=2, space="PSUM") as psA,
        ):
            ident = cpool.tile([128, 128], F32, tag="ident")
            nc.sync.dma_start(ident[:], ident_in[:])

            # Wcat_l = [W_l | W_l@a_src | W_l@a_dst], plus bias broadcast
            wcat = []
            s_dst_res = []
            for l, di in ((1, IN), (2, HID), (3, HID)):
                w_sb = cpool.tile([di, HID], F32, tag=f"w{l}")
                nc.sync.dma_start(w_sb[:], Ws[f"W{l}"][:])
                wc = cpool.tile([di, HID + 2], F32, tag=f"wc{l}")
                nc.vector.tensor_copy(wc[:, :HID], w_sb[:])
                if NOPRO:
                    nc.vector.memset(wc[:, HID:], 0.01)
                else:
                    ps_wt = psA.tile([HID, 128], F32, space="PSUM", tag="aux", name="ps_wt")
                    nc.tensor.transpose(ps_wt[:, :di], w_sb[:], ident[:di, :di])
                    wt_sb = cpool.tile([HID, 128], F32, tag=f"wt{l}")
                    nc.scalar.copy(wt_sb[:, :di], ps_wt[:, :di])
                    for name, col in ((f"as{l}", HID), (f"ad{l}", HID + 1)):
                        a_sb = cpool.tile([HID, 1], F32, tag=f"t{name}")
                        nc.sync.dma_start(a_sb[:], Ws[name][:])
                        ps_wa = psA.tile([128, 1], F32, space="PSUM", tag="aux", name="ps_wa")
                        nc.tensor.matmul(ps_wa[:di, :], wt_sb[:, :di], a_sb[:])
                        nc.vector.tensor_copy(wc[:, col:col + 1], ps_wa[:di, :])
                wcat.append(wc)
                b_sb = cpool.tile([128, HID], F32, tag=f"bb{l}")
                nc.sync.dma_start(b_sb[:], Ws[f"b{l}"][:])
                Ws[f"bsb{l}"] = b_sb
                s_dst_res.append(cpool.tile([128, NT], F32, tag=f"sdst{l}", name=f"sdst{l}"))

            vld_sb = cpool.tile([128, NT], F32, tag="vld")
            nc.sync.dma_start(vld_sb[:], vld_t[:])

            # phase A, layer 1
            for t in range(NT):
                x_sb = spool.tile([IN, 128], F32, tag="ax")
                nc.sync.dma_start(x_sb[:], xT[:, t * 128:(t + 1) * 128])
                ps_h = psA.tile([128, HID + 2], F32, space="PSUM", tag="ph", name="ps_h")
                nc.tensor.matmul(ps_h[:], x_sb[:], wcat[0][:])
                hx = spool.tile([128, ROW], BF16, tag="hx")
                nc.vector.memset(hx[:, HID + 1:], 0.0)
                nc.scalar.copy(hx[:, :HID + 1], ps_h[:, :HID + 1])
                nc.vector.tensor_copy(s_dst_res[0][:, t:t + 1],
                                      ps_h[:, HID + 1:HID + 2])
                nc.sync.dma_start(agin[0][t * 128:(t + 1) * 128, :], hx[:])

            # 3 GAT layers
            nlayers = 3 if DBG == 0 else 1
            for l in range(nlayers):
                if NOCC:
                    nc.sync.dma_start(htab[l][0:NPADC, :], agin[l][:])
                else:
                    nc.gpsimd.collective_compute(
                        "AllGather", ALU.bypass,
                        replica_groups=[list(range(NCORES))],
                        ins=[agin[l][:]], outs=[htab[l][:]],
                    )
                for ii, (ts, T, dc) in enumerate(items):
                    if DBG == 1 or ii >= MAXITEMS:
                        break
                    S = int(dc.sum())
                    G_sb = gpool.tile([128, 128, ROW], BF16, tag="G")
                    goff = G_sb[:].offset
                    offd = 0
                    for chn in range(min(NCH, MAXCH)):
                        D = int(dc[chn])
                        cbase, ncols, ni = idx_cols[ii][chn]
                        rows_c = min(CHUNK, NTOT - chn * CHUNK)
                        ix = spool.tile([128, ncols], I16, tag="ix",
                                        padded_shape=[128, 2048])
                        nc.sync.dma_start(ix[:],
                                          idx_t[:, cbase:cbase + ncols])
                        in_ap = _ap(htab[l], chn * CHUNK * ROW,
                                    [(ROW, rows_c), (1, ROW)])
                        for off in range(0, ni, GMAX):
                            sni = min(GMAX, ni - off)
                            out_ap = _ap(
                                G_sb.tensor,
                                goff + (offd * T + off // 128) * ROW,
                                [(128 * ROW, 128), (ROW, sni // 128), (1, ROW)])
                            nc.gpsimd.dma_gather(
                                out_ap, in_ap,
                                ix[:, off // 16:(off + sni) // 16],
                                sni, sni, ROW, single_packet=False)
                        offd += D
                    if DBG == 2:
                        continue
                    mbase = msk_cols[ii]
                    mk = spool.tile([128, 128], F32, tag="mk")
                    nc.sync.dma_start(mk[:, :T * S],
                                      msk_t[:, mbase:mbase + T * S])
                    mtv = _ap(mk.tensor, mk[:].offset,
                              [(128, 128), (S, T), (1, S)])
                    ssv = _ap(G_sb.tensor, goff + HID,
                              [(128 * ROW, 128), (T * ROW, S), (ROW, T)])
                    e_sb = spool.tile([128, 128], F32, tag="e")
                    ev = _ap(e_sb.tensor, e_sb[:].offset,
                             [(128, 128), (1, S), (S, T)])
                    nc.vector.tensor_copy(ev, ssv)
                    et = _ap(e_sb.tensor, e_sb[:].offset,
                             [(128, 128), (S, T), (1, S)])
                    nc.vector.tensor_tensor(et, et, mtv, ALU.add)
                    sdv = _ap(s_dst_res[l].tensor, s_dst_res[l][:].offset + ts,
                              [(NT, 128), (1, T), (0, S)])
                    nc.vector.tensor_tensor(et, et, sdv, ALU.add)
                    e2_sb = spool.tile([128, 128], F32, tag="e2")
                    e2t = _ap(e2_sb.tensor, e2_sb[:].offset,
                              [(128, 128), (S, T), (1, S)])
                    nc.scalar.activation(e2t, et, ACTF.Copy, scale=0.2)
                    nc.vector.tensor_tensor(et, et, e2t, ALU.max)
                    red = spool.tile([128, MAX_TILES, 4], F32, tag="red")
                    nc.vector.tensor_reduce(red[:, :T, 0:1], et, AX.X, ALU.max)
                    mxb = _ap(red.tensor, red[:].offset,
                              [(MAX_TILES * 4, 128), (4, T), (0, S)])
                    nc.vector.tensor_tensor(et, et, mxb, ALU.subtract)
                    nc.scalar.activation(et, et, ACTF.Exp)
                    nc.vector.tensor_reduce(red[:, :T, 1:2], et, AX.X, ALU.add)
                    nc.vector.reciprocal(red[:, :T, 2:3], red[:, :T, 1:2])
                    nb = spool.tile([128, 128], BF16, tag="nb")
                    nbt = _ap(nb.tensor, nb[:].offset,
                              [(128, 128), (S, T), (1, S)])
                    nc.vector.tensor_copy(nbt, et)
                    # P[t][j][f] = G_h * num
                    P_sb = ppool.tile([128, 128, HID], BF16, tag="P")
                    poff = P_sb[:].offset
                    ghv = _ap(G_sb.tensor, goff,
                              [(128 * ROW, 128), (T * ROW, S), (ROW, T), (1, HID)])
                    nbv = _ap(nb.tensor, nb[:].offset,
                              [(128, 128), (1, S), (S, T), (0, HID)])
                    pv = _ap(P_sb.tensor, poff,
                             [(128 * HID, 128), (HID, S), (S * HID, T), (1, HID)])
                    nc.any.tensor_tensor(pv, ghv, nbv, ALU.mult)
                    o_sb = spool.tile([128, MAX_TILES, HID], F32, tag="o")
                    prd = _ap(P_sb.tensor, poff,
                              [(128 * HID, 128), (S * HID, T), (1, HID), (HID, S)])
                    nc.vector.tensor_reduce(o_sb[:, :T, :], prd, AX.X, ALU.add)
                    rdb = _ap(red.tensor, red[:].offset + 2,
                              [(MAX_TILES * 4, 128), (4, T), (0, HID)])
                    nc.vector.tensor_tensor(o_sb[:, :T, :], o_sb[:, :T, :],
                                            rdb, ALU.mult)
                    bb = _ap(Ws[f"bsb{l + 1}" if l < 2 else "bsb3"].tensor,
                             Ws[f"bsb{l + 1}" if l < 2 else "bsb3"][:].offset,
                             [(HID, 128), (0, T), (1, HID)])
                    nc.vector.tensor_tensor(o_sb[:, :T, :], o_sb[:, :T, :],
                                            bb, ALU.add)
                    nc.scalar.activation(o_sb[:, :T, :], o_sb[:, :T, :],
                                         ACTF.Relu)
                    if l == 2:
                        vb = _ap(vld_sb.tensor, vld_sb[:].offset + ts,
                                 [(NT, 128), (1, T), (0, HID)])
                        nc.vector.tensor_tensor(o_sb[:, :T, :], o_sb[:, :T, :],
                                                vb, ALU.mult)
                    if DBG == 3:
                        continue
                    for t in range(T):
                        ps_t = pspool.tile([HID, 128], F32, space="PSUM")
                        nc.tensor.transpose(ps_t[:], o_sb[:, t, :], ident[:])
                        xt_sb = spool.tile([HID, 128], F32, tag="xt")
                        nc.scalar.copy(xt_sb[:], ps_t[:])
                        if l < 2:
                            ps_h = psA.tile([128, HID + 2], F32, space="PSUM", tag="ph", name="ps_h")
                            nc.tensor.matmul(ps_h[:], xt_sb[:], wcat[l + 1][:])
                            hx = spool.tile([128, ROW], BF16, tag="hx")
                            nc.vector.memset(hx[:, HID + 1:], 0.0)
                            nc.scalar.copy(hx[:, :HID + 1], ps_h[:, :HID + 1])
                            nc.vector.tensor_copy(
                                s_dst_res[l + 1][:, ts + t:ts + t + 1],
                                ps_h[:, HID + 1:HID + 2])
                            nc.sync.dma_start(
                                agin[l + 1][(ts + t) * 128:(ts + t + 1) * 128, :],
                                hx[:])
                        else:
                            nc.sync.dma_start(
                                x4T_d[:, (ts + t) * 128:(ts + t + 1) * 128],
                                xt_sb[:])

            # head
            if DBG:
                o_dbg = cpool.tile([gper, 1], F32, tag="odbg")
                nc.vector.memset(o_dbg[:], 0.5)
                nc.sync.dma_start(out_t[:], o_dbg[:])
            hmaxT = cpool.tile([HID, gper], F32, tag="hmaxT")
            if DBG:
                hmaxT = None
            for g in range(gper if not DBG else 0):
                hg = spool.tile([HID, GLMAX], F32, tag="hg")
                nc.sync.dma_start(hg[:, :GL[g]], x4T_d[:, GST[g]:GST[g] + GL[g]])
                nc.vector.tensor_reduce(hmaxT[:, g:g + 1], hg[:, :GL[g]],
                                        AX.X, ALU.max)
            if not DBG:
                lw_sb = cpool.tile([HID, HID], F32, tag="l0w")
                nc.sync.dma_start(lw_sb[:], lin0W[:])
                ps_g = psA.tile([gper, HID], F32, space="PSUM", tag="aux", name="ps_g")
                nc.tensor.matmul(ps_g[:], hmaxT[:], lw_sb[:])
                b0_sb = cpool.tile([gper, HID], F32, tag="l0b")
                nc.sync.dma_start(b0_sb[:], lin0b[:])
                h0 = cpool.tile([gper, HID], F32, tag="h0")
                nc.vector.tensor_tensor(h0[:], ps_g[:], b0_sb[:], ALU.add)
                nc.scalar.activation(h0[:], h0[:], ACTF.Relu)

                xr_sb = cpool.tile([IN, gper], F32, tag="xr")
                nc.sync.dma_start(xr_sb[:], xrootT[:])
                lnw_sb = cpool.tile([IN, HID], F32, tag="lnw")
                nc.sync.dma_start(lnw_sb[:], linnW[:])
                ps_n = psA.tile([gper, HID], F32, space="PSUM", tag="aux", name="ps_n")
                nc.tensor.matmul(ps_n[:], xr_sb[:], lnw_sb[:])
                bn_sb = cpool.tile([gper, HID], F32, tag="lnb")
                nc.sync.dma_start(bn_sb[:], linnb[:])
                hn = cpool.tile([gper, HID], F32, tag="hn")
                nc.vector.tensor_tensor(hn[:], ps_n[:], bn_sb[:], ALU.add)
                nc.scalar.activation(hn[:], hn[:], ACTF.Relu)

                catT = cpool.tile([2 * HID, gper], F32, tag="catT")
                ps_t0 = psA.tile([HID, gper], F32, space="PSUM", tag="aux", name="ps_t0")
                nc.tensor.transpose(ps_t0[:], h0[:], ident[:gper, :gper])
                nc.scalar.copy(catT[:HID, :], ps_t0[:])
                ps_t1 = psA.tile([HID, gper], F32, space="PSUM", tag="aux", name="ps_t1")
                nc.tensor.transpose(ps_t1[:], hn[:], ident[:gper, :gper])
                nc.scalar.copy(catT[HID:, :], ps_t1[:])

                l1w_sb = cpool.tile([2 * HID, 1], F32, tag="l1w")
                nc.sync.dma_start(l1w_sb[:], lin1W[:])
                ps_o = psA.tile([gper, 1], F32, space="PSUM", tag="aux", name="ps_o")
                nc.tensor.matmul(ps_o[:], catT[:], l1w_sb[:])
                b1_sb = cpool.tile([gper, 1], F32, tag="l1b")
                nc.sync.dma_start(b1_sb[:], lin1b[:])
                o_fin = cpool.tile([gper, 1], F32, tag="ofin")
                nc.scalar.activation(o_fin[:], ps_o[:], ACTF.Sigmoid, bias=b1_sb[:])
                nc.sync.dma_start(out_t[:], o_fin[:])

    nc.compile()
    return nc


# ----------------------------------------------------------------------------
# entry point
# ----------------------------------------------------------------------------

_CACHE = {}
LAST_RESULTS = None
LAST_NC = None
LAST_INMAPS = None


def kernel(x, adj, batch, W1, a_src1, a_dst1, b1, W2, a_src2, a_dst2, b2,
           W3, a_src3, a_dst3, b3, linnews_W, linnews_b, lin0_W, lin0_b,
           lin1_W, lin1_b):
    x = np.asarray(x)
    adj = np.asarray(adj)
    batch = np.asarray(batch)
    N, IN = x.shape
    HID = np.asarray(W1).shape[1]
    gper = NGRAPH // NCORES

    ckey = (N, adj.shape[1], IN, HID,
            hash(adj.tobytes()), hash(batch.tobytes()))
    if ckey in _CACHE:
        pp, nc = _CACHE[ckey]
    else:
        pp = _preprocess(adj, batch)
        nc = _build_program(pp, IN, HID)
        _CACHE.clear()
        _CACHE[ckey] = (pp, nc)

    NPADC = pp["NPADC"]
    order_padded = pp["order_padded"]
    f32 = np.float32
    in_maps = []
    for c in range(NCORES):
        oc = order_padded[c]
        xc = np.zeros((NPADC, IN), f32)
        real = oc >= 0
        xc[real] = np.asarray(x, f32)[oc[real]]
        roots = pp["roots"][c * gper:(c + 1) * gper]
        im = {
            "xT": np.ascontiguousarray(xc.T),
            "xrootT": np.ascontiguousarray(np.asarray(x, f32)[roots].T),
            "idx": pp["idx_tabs"][c],
            "msk": pp["msk_tabs"][c],
            "vld": pp["vlds"][c],
            "W1": np.asarray(W1, f32), "W2": np.asarray(W2, f32),
            "W3": np.asarray(W3, f32),
            "as1": np.asarray(a_src1, f32).reshape(HID, 1),
            "ad1": np.asarray(a_dst1, f32).reshape(HID, 1),
            "as2": np.asarray(a_src2, f32).reshape(HID, 1),
            "ad2": np.asarray(a_dst2, f32).reshape(HID, 1),
            "as3": np.asarray(a_src3, f32).reshape(HID, 1),
            "ad3": np.asarray(a_dst3, f32).reshape(HID, 1),
            "b1": np.tile(np.asarray(b1, f32).reshape(1, HID), (128, 1)),
            "b2": np.tile(np.asarray(b2, f32).reshape(1, HID), (128, 1)),
            "b3": np.tile(np.asarray(b3, f32).reshape(1, HID), (128, 1)),
            "lin0W": np.asarray(lin0_W, f32),
            "lin0b": np.tile(np.asarray(lin0_b, f32).reshape(1, HID), (gper, 1)),
            "linnW": np.asarray(linnews_W, f32),
            "linnb": np.tile(np.asarray(linnews_b, f32).reshape(1, HID),
                             (gper, 1)),
            "lin1W": np.asarray(lin1_W, f32).reshape(2 * HID, 1),
            "lin1b": np.tile(np.asarray(lin1_b, f32).reshape(1, 1), (gper, 1)),
            "ident": np.eye(128, dtype=f32),
        }
        in_maps.append(im)

    kw = {}
    if os.environ.get("GAT_TRACE", "0") == "1":
        kw = dict(trace=True)
    global LAST_RESULTS, LAST_NC, LAST_INMAPS
    LAST_NC, LAST_INMAPS = nc, in_maps
    res = run_bass_kernel_spmd(nc, in_maps, core_ids=list(range(NCORES)), **kw)
    LAST_RESULTS = res
    out = np.concatenate([res.results[c]["out"] for c in range(NCORES)], axis=0)
    return out.astype(np.float32)



# revision 6
# speedup vs baseline: 1.5643x; 1.5643x over previous
"""GAT (3 layers, heads=1) + global-max-pool + MLP head on 8 Trainium2 NeuronCores.

Sharding: 64 graphs -> 8 cores (8 graphs each; batch is sorted so graphs are
contiguous node ranges).  Graph slot j on every core is padded to a common
length GL[j] so all cores run one identical NEFF (SPMD).  Within a graph,
nodes are sorted by descending in-degree (tightens padded-CSR slot grids).

Per layer: each core computes h_ext = [h | h@a_src] rows (bf16, 256B) for its
own nodes, AllGathers the table to every core, then for each work item
(group of 128-node tiles) gathers the neighbor rows of its own edges with
dma_gather.  Indices are int16, so the table is addressed in 32768-row
windows: each node's neighbor list is split by window, and the slot grid has
per-(item, window) padded depth D_c.  Masked stable segment softmax and the
weighted feature sum run on DVE/ACT; the PE transposes each output tile and
applies the next layer's [W | W@a_src | W@a_dst] in a fused tail.

Performance state (2026-08-05): real problem rel-err 3.0e-4; wall 98.9 ms/call
of which ~74 ms is the axon PJRT dispatch floor -> device time ~25 ms.
TimelineSim (cost model, single-core, GAT_NOCC=1) predicts only 2.9 ms, so
~22 ms is unmodeled: prime suspects are the 3 AllGathers (27 MB each, absent
from the model run), SWDGE descriptor-ring writes for ~420k gather
descriptors/layer, and per-packet SDMA overhead with single_packet=False.
Measured via isolated probes (probe_ag.py / probe_gather.py, device-resident
timing minus the 74 ms floor):
- AllGather 27 MB x8 cores: ~1.95 ms each -> the 3 AGs = ~5.8 ms of the 25.
- dma_gather, single_packet=False, 4096-idx calls: ~50 ns/row (~21.5 ms for
  one layer's 426k rows in isolation) vs the 1.4 ns/row cost model -- the
  gather path is the dominant bottleneck (~16 ms of the 25 after overlap).
- single_packet=True with 512-idx calls (33 descs/engine, within the 64/pkt
  limit) should restore the fast path BUT 832 back-to-back calls crash the
  device: the SWDGE ring (16384 descs, ~1024/engine-lane) overflows without
  flow control.  Fix: bound outstanding gathers to <= ~25 calls (ring/33)
  via consuming ops or explicit sem waits, then re-measure.
Next steps, in order of expected win: (1) packetized 512-idx gathers with
flow control (potential ~16 -> ~4 ms); (2) overlap/chunk the AllGathers
(~5.8 ms, can hide under phase-B tails); (3) per-window degree-sorted grids
+ dma_scatter_add combine to cut the ~2.5x slot padding.
"""

import os
import sys
import numpy as np

DBG = int(os.environ.get("GAT_DBG", "0"))
MAXITEMS = int(os.environ.get("GAT_MAXITEMS", "9999"))
MAXCH = int(os.environ.get("GAT_MAXCH", "9999"))
NOPRO = int(os.environ.get("GAT_NOPRO", "0"))
NOCC = int(os.environ.get("GAT_NOCC", "0"))

for _p in ("/opt/trn_rl_repo", "/opt/trn_rl_repo/concourse"):
    if _p not in sys.path:
        sys.path.insert(0, _p)

import concourse.bass as bass  # noqa: E402
import concourse.bacc as bacc  # noqa: E402
import concourse.mybir as mybir  # noqa: E402
import concourse.tile as tile  # noqa: E402
from concourse import library_config  # noqa: E402
from concourse.masks import make_identity  # noqa: E402
from concourse.bass_utils import run_bass_kernel_spmd  # noqa: E402

F32 = mybir.dt.float32
BF16 = mybir.dt.bfloat16
I16 = mybir.dt.int16
ALU = mybir.AluOpType
ACTF = mybir.ActivationFunctionType
AX = mybir.AxisListType

NCORES = 8
NGRAPH = 64
CHUNK = 32768          # int16 index reach per dma_gather call
ROW = 128              # bf16 values per h_ext row = 256B
# SLOT_BUDGET bounds calls-in-flight: with gpool bufs=2, at most
# 2*(SLOT_BUDGET/GMAX)=24 packetized gathers are outstanding, each <=33
# descs/engine-lane -- safely under the SWDGE ring capacity (~1024/lane).
SLOT_BUDGET = 6144     # max gathered slots per work item
MAX_TILES = 16
NEG = -1.0e30
GMAX = 512          # max idxs per dma_gather call (64 descs/lane packet bound)
GCOLS = SLOT_BUDGET // 128   # G-buffer columns per partition


def _ap(t, off, dims):
    return bass.AP(t, off, dims)


# ----------------------------------------------------------------------------
# Host-side preprocessing (sharding / layout + static CSR tables)
# ----------------------------------------------------------------------------

def _preprocess(adj, batch):
    N = batch.shape[0]
    gper = NGRAPH // NCORES
    graph_of = batch.astype(np.int64)
    counts = np.bincount(graph_of, minlength=NGRAPH)
    gstarts = np.zeros(NGRAPH + 1, np.int64)
    np.cumsum(counts, out=gstarts[1:])

    src = np.concatenate([adj[0].astype(np.int64), np.arange(N, dtype=np.int64)])
    dst = np.concatenate([adj[1].astype(np.int64), np.arange(N, dtype=np.int64)])
    deg = np.bincount(dst, minlength=N)

    # common padded per-graph-slot lengths
    glens = counts.reshape(NCORES, gper)
    GL = np.maximum(glens.max(axis=0), 1)            # [gper]
    GST = np.zeros(gper + 1, np.int64)
    np.cumsum(GL, out=GST[1:])
    NPADC = int(np.ceil(GST[-1] / 128) * 128)
    NT = NPADC // 128

    # per-graph degree-desc order; old -> new id (new = core*NPADC + col)
    order = np.lexsort((-deg, graph_of))             # old ids, grouped by graph
    new_of_old = np.empty(N, np.int64)
    order_padded = np.full((NCORES, NPADC), -1, np.int64)
    for g in range(NGRAPH):
        c, j = g // gper, g % gper
        olds = order[gstarts[g]:gstarts[g + 1]]
        col0 = GST[j]
        order_padded[c, col0:col0 + len(olds)] = olds
        new_of_old[olds] = c * NPADC + col0 + np.arange(len(olds))

    NTOT = NCORES * NPADC
    NCH = int((NTOT + CHUNK - 1) // CHUNK)

    nsrc = new_of_old[src]
    ndst = new_of_old[dst]
    dst_core = ndst // NPADC
    dst_local = ndst % NPADC
    ch_src = nsrc // CHUNK
    loc_src = (nsrc % CHUNK).astype(np.int32)

    # per-(core, local node, chunk) degree; max over cores
    degc = np.zeros((NCORES, NPADC, NCH), np.int32)
    np.add.at(degc, (dst_core, dst_local, ch_src), 1)
    degc_max = degc.max(axis=0)

    # shared work-item schedule
    tile_dc = degc_max.reshape(NT, 128, NCH).max(axis=1)
    items = []
    t0 = 0
    while t0 < NT:
        T = 1
        dcur = np.maximum(tile_dc[t0], 1)
        while t0 + T < NT and T < MAX_TILES:
            nd = np.maximum(np.maximum(dcur, tile_dc[t0 + T]), 1)
            if (T + 1) * 128 * int(nd.sum()) > SLOT_BUDGET:
                break
            dcur = nd
            T += 1
        items.append((t0, T, dcur.copy()))
        t0 += T

    # table layouts
    idx_cols = []   # per item: per chunk (col_off, ncols, num_idx)
    msk_cols = []   # per item: col_off
    icol = mcol = 0
    for (ts, T, dc) in items:
        S = int(dc.sum())
        msk_cols.append(mcol)
        mcol += T * S
        row = []
        for c in range(NCH):
            ni = 128 * T * int(dc[c])
            row.append((icol, ni // 16, ni))
            icol += ni // 16
        idx_cols.append(row)
    IDXCOLS, MSKCOLS = icol, mcol

    # per-node lookup arrays for vectorized fill
    item_of_tile = np.zeros(NT, np.int64)
    for ii, (ts, T, dc) in enumerate(items):
        item_of_tile[ts:ts + T] = ii
    arr_ts = np.array([it[0] for it in items], np.int64)
    arr_T = np.array([it[1] for it in items], np.int64)
    arr_S = np.array([int(it[2].sum()) for it in items], np.int64)
    arr_offd = np.zeros((len(items), NCH), np.int64)
    arr_cbase = np.zeros((len(items), NCH), np.int64)
    for ii in range(len(items)):
        off = 0
        for c in range(NCH):
            arr_offd[ii, c] = off
            arr_cbase[ii, c] = idx_cols[ii][c][0]
            off += int(items[ii][2][c])
    arr_mbase = np.array(msk_cols, np.int64)

    idx_tabs, msk_tabs, vlds = [], [], []
    for c in range(NCORES):
        m = dst_core == c
        o = np.lexsort((loc_src[m], ch_src[m], dst_local[m]))
        dl = dst_local[m][o]
        ch = ch_src[m][o]
        lo = loc_src[m][o]
        ne = len(dl)
        # rank within (node, chunk)
        if ne:
            keys = dl * NCH + ch
            brk = np.ones(ne, bool)
            brk[1:] = keys[1:] != keys[:-1]
            gid = np.cumsum(brk) - 1
            gst = np.zeros(gid[-1] + 2 if ne else 1, np.int64)
            np.add.at(gst[1:], gid, 1)
            np.cumsum(gst, out=gst)
            rank = np.arange(ne) - gst[gid]
        else:
            rank = np.zeros(0, np.int64)
        til = dl // 128
        p = dl % 128
        ii = item_of_tile[til]
        t = til - arr_ts[ii]
        T = arr_T[ii]
        S = arr_S[ii]
        D = items[0][2]  # placeholder
        # gather idx table
        q = rank * (T * 128) + t * 128 + p
        col = arr_cbase[ii, ch] + q // 16
        rrow = q % 16
        it = np.zeros((16, IDXCOLS), np.int16)
        it[rrow, col] = lo.astype(np.int16)
        idx_tabs.append(np.tile(it, (8, 1)))
        # mask table
        mt = np.full((128, MSKCOLS), NEG, np.float32)
        mcolv = arr_mbase[ii] + t * S + arr_offd[ii, ch] + rank
        mt[p, mcolv] = 0.0
        # pad nodes: unmask slot (chunk0, j=0) so den=1
        vld = np.zeros((128, NT), np.float32)
        padm = order_padded[c] < 0
        for ti in range(NT):
            iii = item_of_tile[ti]
            tt = ti - arr_ts[iii]
            SS = arr_S[iii]
            prow = np.nonzero(padm[ti * 128:(ti + 1) * 128])[0]
            mt[prow, arr_mbase[iii] + tt * SS] = 0.0
            vld[:, ti] = (~padm[ti * 128:(ti + 1) * 128]).astype(np.float32)
        msk_tabs.append(mt)
        vlds.append(vld)

    return dict(
        N=N, gper=gper, NPADC=NPADC, NT=NT, NTOT=NTOT, NCH=NCH,
        order_padded=order_padded, items=items,
        idx_cols=idx_cols, msk_cols=msk_cols,
        IDXCOLS=IDXCOLS, MSKCOLS=MSKCOLS,
        idx_tabs=idx_tabs, msk_tabs=msk_tabs, vlds=vlds,
        GL=[int(v) for v in GL], GST=[int(v) for v in GST],
        roots=gstarts[:NGRAPH].copy(),
    )


# ----------------------------------------------------------------------------
# Device program
# ----------------------------------------------------------------------------

def _build_program(pp, IN, HID):
    NPADC, NT, NTOT, NCH = pp["NPADC"], pp["NT"], pp["NTOT"], pp["NCH"]
    items, idx_cols, msk_cols = pp["items"], pp["idx_cols"], pp["msk_cols"]
    IDXCOLS, MSKCOLS = pp["IDXCOLS"], pp["MSKCOLS"]
    GL, GST, gper = pp["GL"], pp["GST"], pp["gper"]
    GLMAX = int(np.ceil(max(GL) / 128) * 128)
    # actual G-buffer columns: a lone high-degree tile may exceed SLOT_BUDGET
    GC = max(T * int(dc.sum()) for (_, T, dc) in items)
    IXPAD = max(ncols for row in idx_cols for (_, ncols, _) in row)

    nc = bacc.Bacc("TRN2", target_bir_lowering=False, debug=False,
                   num_devices=(1 if NOCC else NCORES))

    xT = nc.dram_tensor("xT", [IN, NPADC], F32, kind="ExternalInput")
    xrootT = nc.dram_tensor("xrootT", [IN, gper], F32, kind="ExternalInput")
    idx_t = nc.dram_tensor("idx", [128, IDXCOLS], I16, kind="ExternalInput")
    msk_t = nc.dram_tensor("msk", [128, MSKCOLS], F32, kind="ExternalInput")
    vld_t = nc.dram_tensor("vld", [128, NT], F32, kind="ExternalInput")
    Ws = {}
    for l, di in ((1, IN), (2, HID), (3, HID)):
        Ws[f"W{l}"] = nc.dram_tensor(f"W{l}", [di, HID], F32, kind="ExternalInput")
        Ws[f"as{l}"] = nc.dram_tensor(f"as{l}", [HID, 1], F32, kind="ExternalInput")
        Ws[f"ad{l}"] = nc.dram_tensor(f"ad{l}", [HID, 1], F32, kind="ExternalInput")
        Ws[f"b{l}"] = nc.dram_tensor(f"b{l}", [128, HID], F32, kind="ExternalInput")
    lin0W = nc.dram_tensor("lin0W", [HID, HID], F32, kind="ExternalInput")
    lin0b = nc.dram_tensor("lin0b", [gper, HID], F32, kind="ExternalInput")
    linnW = nc.dram_tensor("linnW", [IN, HID], F32, kind="ExternalInput")
    linnb = nc.dram_tensor("linnb", [gper, HID], F32, kind="ExternalInput")
    lin1W = nc.dram_tensor("lin1W", [2 * HID, 1], F32, kind="ExternalInput")
    lin1b = nc.dram_tensor("lin1b", [gper, 1], F32, kind="ExternalInput")
    ident_in = nc.dram_tensor("ident", [128, 128], F32, kind="ExternalInput")
    out_t = nc.dram_tensor("out", [gper, 1], F32, kind="ExternalOutput")

    agin = [nc.dram_tensor(f"agin{l}", [NPADC, ROW], BF16, kind="Internal")
            for l in range(3)]
    htab = [nc.dram_tensor(f"htab{l}", [NTOT, ROW], BF16, kind="Internal",
                           addr_space="Shared")
            for l in range(3)]
    x4T_d = nc.dram_tensor("x4T", [HID, NPADC], F32, kind="Internal")

    with tile.TileContext(nc) as tc:
        with (
            tc.tile_pool(name="const", bufs=1) as cpool,
            tc.tile_pool(name="gbuf", bufs=2) as gpool,
            tc.tile_pool(name="pbuf", bufs=2) as ppool,
            tc.tile_pool(name="sbuf", bufs=3) as spool,
            tc.tile_pool(name="psum", bufs=2, space="PSUM") as pspool,
            tc.tile_pool(name="psA", bufs=2, space="PSUM") as psA,
        ):
            ident = cpool.tile([128, 128], F32, tag="ident")
            nc.sync.dma_start(ident[:], ident_in[:])

            # Wcat_l = [W_l | W_l@a_src | W_l@a_dst], plus bias broadcast
            wcat = []
            s_dst_res = []
            for l, di in ((1, IN), (2, HID), (3, HID)):
                w_sb = cpool.tile([di, HID], F32, tag=f"w{l}")
                nc.sync.dma_start(w_sb[:], Ws[f"W{l}"][:])
                wc = cpool.tile([di, HID + 2], F32, tag=f"wc{l}")
                nc.vector.tensor_copy(wc[:, :HID], w_sb[:])
                if NOPRO:
                    nc.vector.memset(wc[:, HID:], 0.01)
                else:
                    ps_wt = psA.tile([HID, 128], F32, space="PSUM", tag="aux", name="ps_wt")
                    nc.tensor.transpose(ps_wt[:, :di], w_sb[:], ident[:di, :di])
                    wt_sb = cpool.tile([HID, 128], F32, tag=f"wt{l}")
                    nc.scalar.copy(wt_sb[:, :di], ps_wt[:, :di])
                    for name, col in ((f"as{l}", HID), (f"ad{l}", HID + 1)):
                        a_sb = cpool.tile([HID, 1], F32, tag=f"t{name}")
                        nc.sync.dma_start(a_sb[:], Ws[name][:])
                        ps_wa = psA.tile([128, 1], F32, space="PSUM", tag="aux", name="ps_wa")
                        nc.tensor.matmul(ps_wa[:di, :], wt_sb[:, :di], a_sb[:])
                        nc.vector.tensor_copy(wc[:, col:col + 1], ps_wa[:di, :])
                wcat.append(wc)
                b_sb = cpool.tile([128, HID], F32, tag=f"bb{l}")
                nc.sync.dma_start(b_sb[:], Ws[f"b{l}"][:])
                Ws[f"bsb{l}"] = b_sb
                s_dst_res.append(cpool.tile([128, NT], F32, tag=f"sdst{l}", name=f"sdst{l}"))

            vld_sb = cpool.tile([128, NT], F32, tag="vld")
            nc.sync.dma_start(vld_sb[:], vld_t[:])

            # phase A, layer 1
            for t in range(NT):
                x_sb = spool.tile([IN, 128], F32, tag="ax")
                nc.sync.dma_start(x_sb[:], xT[:, t * 128:(t + 1) * 128])
                ps_h = psA.tile([128, HID + 2], F32, space="PSUM", tag="ph", name="ps_h")
                nc.tensor.matmul(ps_h[:], x_sb[:], wcat[0][:])
                hx = spool.tile([128, ROW], BF16, tag="hx")
                nc.vector.memset(hx[:, HID + 1:], 0.0)
                nc.scalar.copy(hx[:, :HID + 1], ps_h[:, :HID + 1])
                nc.vector.tensor_copy(s_dst_res[0][:, t:t + 1],
                                      ps_h[:, HID + 1:HID + 2])
                nc.sync.dma_start(agin[0][t * 128:(t + 1) * 128, :], hx[:])

            # 3 GAT layers
            nlayers = 3 if DBG == 0 else 1
            for l in range(nlayers):
                if NOCC:
                    nc.sync.dma_start(htab[l][0:NPADC, :], agin[l][:])
                else:
                    nc.gpsimd.collective_compute(
                        "AllGather", ALU.bypass,
                        replica_groups=[list(range(NCORES))],
                        ins=[agin[l][:]], outs=[htab[l][:]],
                    )
                for ii, (ts, T, dc) in enumerate(items):
                    if DBG == 1 or ii >= MAXITEMS:
                        break
                    S = int(dc.sum())
                    G_sb = gpool.tile([128, GC, ROW], BF16, tag="G")
                    goff = G_sb[:].offset
                    offd = 0
                    for chn in range(min(NCH, MAXCH)):
                        D = int(dc[chn])
                        cbase, ncols, ni = idx_cols[ii][chn]
                        rows_c = min(CHUNK, NTOT - chn * CHUNK)
                        ix = spool.tile([128, ncols], I16, tag="ix",
                                        padded_shape=[128, IXPAD])
                        nc.sync.dma_start(ix[:],
                                          idx_t[:, cbase:cbase + ncols])
                        in_ap = _ap(htab[l], chn * CHUNK * ROW,
                                    [(ROW, rows_c), (1, ROW)])
                        # split ni into even 128-multiple calls <= GMAX so
                        # each call fits one <=33-desc/lane packet
                        ncalls = max(1, -(-ni // GMAX))
                        csz = -(-(ni // 128) // ncalls) * 128
                        off = 0
                        while off < ni:
                            sni = min(csz, ni - off)
                            out_ap = _ap(
                                G_sb.tensor,
                                goff + (offd * T + off // 128) * ROW,
                                [(GC * ROW, 128), (ROW, sni // 128), (1, ROW)])
                            nc.gpsimd.dma_gather(
                                out_ap, in_ap,
                                ix[:, off // 16:(off + sni) // 16],
                                sni, sni, ROW, single_packet=True)
                            off += sni
                        offd += D
                    if DBG == 2:
                        continue
                    mbase = msk_cols[ii]
                    mk = spool.tile([128, 128], F32, tag="mk")
                    nc.sync.dma_start(mk[:, :T * S],
                                      msk_t[:, mbase:mbase + T * S])
                    mtv = _ap(mk.tensor, mk[:].offset,
                              [(128, 128), (S, T), (1, S)])
                    ssv = _ap(G_sb.tensor, goff + HID,
                              [(GC * ROW, 128), (T * ROW, S), (ROW, T)])
                    e_sb = spool.tile([128, 128], F32, tag="e")
                    ev = _ap(e_sb.tensor, e_sb[:].offset,
                             [(128, 128), (1, S), (S, T)])
                    nc.vector.tensor_copy(ev, ssv)
                    et = _ap(e_sb.tensor, e_sb[:].offset,
                             [(128, 128), (S, T), (1, S)])
                    nc.vector.tensor_tensor(et, et, mtv, ALU.add)
                    sdv = _ap(s_dst_res[l].tensor, s_dst_res[l][:].offset + ts,
                              [(NT, 128), (1, T), (0, S)])
                    nc.vector.tensor_tensor(et, et, sdv, ALU.add)
                    e2_sb = spool.tile([128, 128], F32, tag="e2")
                    e2t = _ap(e2_sb.tensor, e2_sb[:].offset,
                              [(128, 128), (S, T), (1, S)])
                    nc.scalar.activation(e2t, et, ACTF.Copy, scale=0.2)
                    nc.vector.tensor_tensor(et, et, e2t, ALU.max)
                    red = spool.tile([128, MAX_TILES, 4], F32, tag="red")
                    nc.vector.tensor_reduce(red[:, :T, 0:1], et, AX.X, ALU.max)
                    mxb = _ap(red.tensor, red[:].offset,
                              [(MAX_TILES * 4, 128), (4, T), (0, S)])
                    nc.vector.tensor_tensor(et, et, mxb, ALU.subtract)
                    nc.scalar.activation(et, et, ACTF.Exp)
                    nc.vector.tensor_reduce(red[:, :T, 1:2], et, AX.X, ALU.add)
                    nc.vector.reciprocal(red[:, :T, 2:3], red[:, :T, 1:2])
                    nb = spool.tile([128, 128], BF16, tag="nb")
                    nbt = _ap(nb.tensor, nb[:].offset,
                              [(128, 128), (S, T), (1, S)])
                    nc.vector.tensor_copy(nbt, et)
                    # P[t][j][f] = G_h * num
                    P_sb = ppool.tile([128, GC, HID], BF16, tag="P")
                    poff = P_sb[:].offset
                    ghv = _ap(G_sb.tensor, goff,
                              [(GC * ROW, 128), (T * ROW, S), (ROW, T), (1, HID)])
                    nbv = _ap(nb.tensor, nb[:].offset,
                              [(128, 128), (1, S), (S, T), (0, HID)])
                    pv = _ap(P_sb.tensor, poff,
                             [(GC * HID, 128), (HID, S), (S * HID, T), (1, HID)])
                    nc.any.tensor_tensor(pv, ghv, nbv, ALU.mult)
                    o_sb = spool.tile([128, MAX_TILES, HID], F32, tag="o")
                    prd = _ap(P_sb.tensor, poff,
                              [(GC * HID, 128), (S * HID, T), (1, HID), (HID, S)])
                    nc.vector.tensor_reduce(o_sb[:, :T, :], prd, AX.X, ALU.add)
                    rdb = _ap(red.tensor, red[:].offset + 2,
                              [(MAX_TILES * 4, 128), (4, T), (0, HID)])
                    nc.vector.tensor_tensor(o_sb[:, :T, :], o_sb[:, :T, :],
                                            rdb, ALU.mult)
                    bb = _ap(Ws[f"bsb{l + 1}" if l < 2 else "bsb3"].tensor,
                             Ws[f"bsb{l + 1}" if l < 2 else "bsb3"][:].offset,
                             [(HID, 128), (0, T), (1, HID)])
                    nc.vector.tensor_tensor(o_sb[:, :T, :], o_sb[:, :T, :],
                                            bb, ALU.add)
                    nc.scalar.activation(o_sb[:, :T, :], o_sb[:, :T, :],
                                         ACTF.Relu)
                    if l == 2:
                        vb = _ap(vld_sb.tensor, vld_sb[:].offset + ts,
                                 [(NT, 128), (1, T), (0, HID)])
                        nc.vector.tensor_tensor(o_sb[:, :T, :], o_sb[:, :T, :],
                                                vb, ALU.mult)
                    if DBG == 3:
                        continue
                    for t in range(T):
                        ps_t = pspool.tile([HID, 128], F32, space="PSUM")
                        nc.tensor.transpose(ps_t[:], o_sb[:, t, :], ident[:])
                        xt_sb = spool.tile([HID, 128], F32, tag="xt")
                        nc.scalar.copy(xt_sb[:], ps_t[:])
                        if l < 2:
                            ps_h = psA.tile([128, HID + 2], F32, space="PSUM", tag="ph", name="ps_h")
                            nc.tensor.matmul(ps_h[:], xt_sb[:], wcat[l + 1][:])
                            hx = spool.tile([128, ROW], BF16, tag="hx")
                            nc.vector.memset(hx[:, HID + 1:], 0.0)
                            nc.scalar.copy(hx[:, :HID + 1], ps_h[:, :HID + 1])
                            nc.vector.tensor_copy(
                                s_dst_res[l + 1][:, ts + t:ts + t + 1],
                                ps_h[:, HID + 1:HID + 2])
                            nc.sync.dma_start(
                                agin[l + 1][(ts + t) * 128:(ts + t + 1) * 128, :],
                                hx[:])
                        else:
                            nc.sync.dma_start(
                                x4T_d[:, (ts + t) * 128:(ts + t + 1) * 128],
                                xt_sb[:])

            # head
            if DBG:
                o_dbg = cpool.tile([gper, 1], F32, tag="odbg")
                nc.vector.memset(o_dbg[:], 0.5)
                nc.sync.dma_start(out_t[:], o_dbg[:])
            hmaxT = cpool.tile([HID, gper], F32, tag="hmaxT")
            if DBG:
                hmaxT = None
            for g in range(gper if not DBG else 0):
                hg = spool.tile([HID, GLMAX], F32, tag="hg")
                nc.sync.dma_start(hg[:, :GL[g]], x4T_d[:, GST[g]:GST[g] + GL[g]])
                nc.vector.tensor_reduce(hmaxT[:, g:g + 1], hg[:, :GL[g]],
                                        AX.X, ALU.max)
            if not DBG:
                lw_sb = cpool.tile([HID, HID], F32, tag="l0w")
                nc.sync.dma_start(lw_sb[:], lin0W[:])
                ps_g = psA.tile([gper, HID], F32, space="PSUM", tag="aux", name="ps_g")
                nc.tensor.matmul(ps_g[:], hmaxT[:], lw_sb[:])
                b0_sb = cpool.tile([gper, HID], F32, tag="l0b")
                nc.sync.dma_start(b0_sb[:], lin0b[:])
                h0 = cpool.tile([gper, HID], F32, tag="h0")
                nc.vector.tensor_tensor(h0[:], ps_g[:], b0_sb[:], ALU.add)
                nc.scalar.activation(h0[:], h0[:], ACTF.Relu)

                xr_sb = cpool.tile([IN, gper], F32, tag="xr")
                nc.sync.dma_start(xr_sb[:], xrootT[:])
                lnw_sb = cpool.tile([IN, HID], F32, tag="lnw")
                nc.sync.dma_start(lnw_sb[:], linnW[:])
                ps_n = psA.tile([gper, HID], F32, space="PSUM", tag="aux", name="ps_n")
                nc.tensor.matmul(ps_n[:], xr_sb[:], lnw_sb[:])
                bn_sb = cpool.tile([gper, HID], F32, tag="lnb")
                nc.sync.dma_start(bn_sb[:], linnb[:])
                hn = cpool.tile([gper, HID], F32, tag="hn")
                nc.vector.tensor_tensor(hn[:], ps_n[:], bn_sb[:], ALU.add)
                nc.scalar.activation(hn[:], hn[:], ACTF.Relu)

                catT = cpool.tile([2 * HID, gper], F32, tag="catT")
                ps_t0 = psA.tile([HID, gper], F32, space="PSUM", tag="aux", name="ps_t0")
                nc.tensor.transpose(ps_t0[:], h0[:], ident[:gper, :gper])
                nc.scalar.copy(catT[:HID, :], ps_t0[:])
                ps_t1 = psA.tile([HID, gper], F32, space="PSUM", tag="aux", name="ps_t1")
                nc.tensor.transpose(ps_t1[:], hn[:], ident[:gper, :gper])
                nc.scalar.copy(catT[HID:, :], ps_t1[:])

                l1w_sb = cpool.tile([2 * HID, 1], F32, tag="l1w")
                nc.sync.dma_start(l1w_sb[:], lin1W[:])
                ps_o = psA.tile([gper, 1], F32, space="PSUM", tag="aux", name="ps_o")
                nc.tensor.matmul(ps_o[:], catT[:], l1w_sb[:])
                b1_sb = cpool.tile([gper, 1], F32, tag="l1b")
                nc.sync.dma_start(b1_sb[:], lin1b[:])
                o_fin = cpool.tile([gper, 1], F32, tag="ofin")
                nc.scalar.activation(o_fin[:], ps_o[:], ACTF.Sigmoid, bias=b1_sb[:])
                nc.sync.dma_start(out_t[:], o_fin[:])

    nc.compile()
    return nc


# ----------------------------------------------------------------------------
# entry point
# ----------------------------------------------------------------------------

_CACHE = {}
LAST_RESULTS = None
LAST_NC = None
LAST_INMAPS = None


def kernel(x, adj, batch, W1, a_src1, a_dst1, b1, W2, a_src2, a_dst2, b2,
           W3, a_src3, a_dst3, b3, linnews_W, linnews_b, lin0_W, lin0_b,
           lin1_W, lin1_b):
    x = np.asarray(x)
    adj = np.asarray(adj)
    batch = np.asarray(batch)
    N, IN = x.shape
    HID = np.asarray(W1).shape[1]
    gper = NGRAPH // NCORES

    ckey = (N, adj.shape[1], IN, HID,
            hash(adj.tobytes()), hash(batch.tobytes()))
    if ckey in _CACHE:
        pp, nc = _CACHE[ckey]
    else:
        pp = _preprocess(adj, batch)
        nc = _build_program(pp, IN, HID)
        _CACHE.clear()
        _CACHE[ckey] = (pp, nc)

    NPADC = pp["NPADC"]
    order_padded = pp["order_padded"]
    f32 = np.float32
    in_maps = []
    for c in range(NCORES):
        oc = order_padded[c]
        xc = np.zeros((NPADC, IN), f32)
        real = oc >= 0
        xc[real] = np.asarray(x, f32)[oc[real]]
        roots = pp["roots"][c * gper:(c + 1) * gper]
        im = {
            "xT": np.ascontiguousarray(xc.T),
            "xrootT": np.ascontiguousarray(np.asarray(x, f32)[roots].T),
            "idx": pp["idx_tabs"][c],
            "msk": pp["msk_tabs"][c],
            "vld": pp["vlds"][c],
            "W1": np.asarray(W1, f32), "W2": np.asarray(W2, f32),
            "W3": np.asarray(W3, f32),
            "as1": np.asarray(a_src1, f32).reshape(HID, 1),
            "ad1": np.asarray(a_dst1, f32).reshape(HID, 1),
            "as2": np.asarray(a_src2, f32).reshape(HID, 1),
            "ad2": np.asarray(a_dst2, f32).reshape(HID, 1),
            "as3": np.asarray(a_src3, f32).reshape(HID, 1),
            "ad3": np.asarray(a_dst3, f32).reshape(HID, 1),
            "b1": np.tile(np.asarray(b1, f32).reshape(1, HID), (128, 1)),
            "b2": np.tile(np.asarray(b2, f32).reshape(1, HID), (128, 1)),
            "b3": np.tile(np.asarray(b3, f32).reshape(1, HID), (128, 1)),
            "lin0W": np.asarray(lin0_W, f32),
            "lin0b": np.tile(np.asarray(lin0_b, f32).reshape(1, HID), (gper, 1)),
            "linnW": np.asarray(linnews_W, f32),
            "linnb": np.tile(np.asarray(linnews_b, f32).reshape(1, HID),
                             (gper, 1)),
            "lin1W": np.asarray(lin1_W, f32).reshape(2 * HID, 1),
            "lin1b": np.tile(np.asarray(lin1_b, f32).reshape(1, 1), (gper, 1)),
            "ident": np.eye(128, dtype=f32),
        }
        in_maps.append(im)

    kw = {}
    if os.environ.get("GAT_TRACE", "0") == "1":
        kw = dict(trace=True)
    global LAST_RESULTS, LAST_NC, LAST_INMAPS
    LAST_NC, LAST_INMAPS = nc, in_maps
    res = run_bass_kernel_spmd(nc, in_maps, core_ids=list(range(NCORES)), **kw)
    LAST_RESULTS = res
    out = np.concatenate([res.results[c]["out"] for c in range(NCORES)], axis=0)
    return out.astype(np.float32)



# revision 7
# speedup vs baseline: 4.6219x; 2.9546x over previous
"""GAT (3 layers, heads=1) + global-max-pool + MLP head on 8 Trainium2 NeuronCores.

Sharding: 64 graphs -> 8 cores (8 graphs each; batch is sorted so graphs are
contiguous node ranges).  Graph slot j on every core is padded to a common
length GL[j] so all cores run one identical NEFF (SPMD).  Within a graph,
nodes are sorted by descending in-degree (tightens padded-CSR slot grids).

Per layer: each core computes h_ext = [h | h@a_src] rows (bf16, 256B) for its
own nodes, AllGathers the table to every core, then for each work item
(group of 128-node tiles) gathers the neighbor rows of its own edges with
dma_gather.  Indices are int16, so the table is addressed in 32768-row
windows: each node's neighbor list is split by window, and the slot grid has
per-(item, window) padded depth D_c.  Masked stable segment softmax and the
weighted feature sum run on DVE/ACT; the PE transposes each output tile and
applies the next layer's [W | W@a_src | W@a_dst] in a fused tail.

Performance state (2026-08-05): real problem rel-err 3.0e-4; wall 98.9 ms/call
of which ~74 ms is the axon PJRT dispatch floor -> device time ~25 ms.
TimelineSim (cost model, single-core, GAT_NOCC=1) predicts only 2.9 ms, so
~22 ms is unmodeled: prime suspects are the 3 AllGathers (27 MB each, absent
from the model run), SWDGE descriptor-ring writes for ~420k gather
descriptors/layer, and per-packet SDMA overhead with single_packet=False.
Measured via isolated probes (probe_ag.py / probe_gather.py, device-resident
timing minus the 74 ms floor):
- AllGather 27 MB x8 cores: ~1.95 ms each -> the 3 AGs = ~5.8 ms of the 25.
- dma_gather, single_packet=False, 4096-idx calls: ~50 ns/row (~21.5 ms for
  one layer's 426k rows in isolation) vs the 1.4 ns/row cost model -- the
  gather path is the dominant bottleneck (~16 ms of the 25 after overlap).
- single_packet=True with 512-idx calls (33 descs/engine, within the 64/pkt
  limit) should restore the fast path BUT 832 back-to-back calls crash the
  device: the SWDGE ring (16384 descs, ~1024/engine-lane) overflows without
  flow control.  Fix: bound outstanding gathers to <= ~25 calls (ring/33)
  via consuming ops or explicit sem waits, then re-measure.
Next steps, in order of expected win: (1) packetized 512-idx gathers with
flow control (potential ~16 -> ~4 ms); (2) overlap/chunk the AllGathers
(~5.8 ms, can hide under phase-B tails); (3) per-window degree-sorted grids
+ dma_scatter_add combine to cut the ~2.5x slot padding.
"""

import os
import sys
import numpy as np

DBG = int(os.environ.get("GAT_DBG", "0"))
MAXITEMS = int(os.environ.get("GAT_MAXITEMS", "9999"))
MAXCH = int(os.environ.get("GAT_MAXCH", "9999"))
NOPRO = int(os.environ.get("GAT_NOPRO", "0"))
NOCC = int(os.environ.get("GAT_NOCC", "0"))

for _p in ("/opt/trn_rl_repo", "/opt/trn_rl_repo/concourse"):
    if _p not in sys.path:
        sys.path.insert(0, _p)

import concourse.bass as bass  # noqa: E402
import concourse.bacc as bacc  # noqa: E402
import concourse.mybir as mybir  # noqa: E402
import concourse.tile as tile  # noqa: E402
from concourse import library_config  # noqa: E402
from concourse.masks import make_identity  # noqa: E402
from concourse.bass_utils import run_bass_kernel_spmd  # noqa: E402

F32 = mybir.dt.float32
BF16 = mybir.dt.bfloat16
I16 = mybir.dt.int16
ALU = mybir.AluOpType
ACTF = mybir.ActivationFunctionType
AX = mybir.AxisListType

NCORES = 8
NGRAPH = 64
CHUNK = 32768          # int16 index reach per dma_gather call
ROW = 128              # bf16 values per h_ext row = 256B
# SLOT_BUDGET bounds calls-in-flight: with gpool bufs=2, at most
# 2*(SLOT_BUDGET/GMAX)=24 packetized gathers are outstanding, each <=33
# descs/engine-lane -- safely under the SWDGE ring capacity (~1024/lane).
SLOT_BUDGET = 6144     # max gathered slots per work item
MAX_TILES = 16
NEG = -1.0e30
GMAX = 512          # max idxs per dma_gather call (64 descs/lane packet bound)
GCOLS = SLOT_BUDGET // 128   # G-buffer columns per partition


def _ap(t, off, dims):
    return bass.AP(t, off, dims)


# ----------------------------------------------------------------------------
# Host-side preprocessing (sharding / layout + static CSR tables)
# ----------------------------------------------------------------------------

def _preprocess(adj, batch):
    N = batch.shape[0]
    gper = NGRAPH // NCORES
    graph_of = batch.astype(np.int64)
    counts = np.bincount(graph_of, minlength=NGRAPH)
    gstarts = np.zeros(NGRAPH + 1, np.int64)
    np.cumsum(counts, out=gstarts[1:])

    src = np.concatenate([adj[0].astype(np.int64), np.arange(N, dtype=np.int64)])
    dst = np.concatenate([adj[1].astype(np.int64), np.arange(N, dtype=np.int64)])
    deg = np.bincount(dst, minlength=N)

    # common padded per-graph-slot lengths
    glens = counts.reshape(NCORES, gper)
    GL = np.maximum(glens.max(axis=0), 1)            # [gper]
    GST = np.zeros(gper + 1, np.int64)
    np.cumsum(GL, out=GST[1:])
    NPADC = int(np.ceil(GST[-1] / 128) * 128)
    NT = NPADC // 128

    # per-graph degree-desc order; old -> new id (new = core*NPADC + col)
    order = np.lexsort((-deg, graph_of))             # old ids, grouped by graph
    new_of_old = np.empty(N, np.int64)
    order_padded = np.full((NCORES, NPADC), -1, np.int64)
    for g in range(NGRAPH):
        c, j = g // gper, g % gper
        olds = order[gstarts[g]:gstarts[g + 1]]
        col0 = GST[j]
        order_padded[c, col0:col0 + len(olds)] = olds
        new_of_old[olds] = c * NPADC + col0 + np.arange(len(olds))

    NTOT = NCORES * NPADC
    NCH = int((NTOT + CHUNK - 1) // CHUNK)

    nsrc = new_of_old[src]
    ndst = new_of_old[dst]
    dst_core = ndst // NPADC
    dst_local = ndst % NPADC
    ch_src = nsrc // CHUNK
    loc_src = (nsrc % CHUNK).astype(np.int32)

    # per-(core, local node, chunk) degree; max over cores
    degc = np.zeros((NCORES, NPADC, NCH), np.int32)
    np.add.at(degc, (dst_core, dst_local, ch_src), 1)
    degc_max = degc.max(axis=0)

    # shared work-item schedule
    tile_dc = degc_max.reshape(NT, 128, NCH).max(axis=1)
    items = []
    t0 = 0
    while t0 < NT:
        T = 1
        dcur = np.maximum(tile_dc[t0], 1)
        while t0 + T < NT and T < MAX_TILES:
            nd = np.maximum(np.maximum(dcur, tile_dc[t0 + T]), 1)
            if (T + 1) * 128 * int(nd.sum()) > SLOT_BUDGET:
                break
            dcur = nd
            T += 1
        items.append((t0, T, dcur.copy()))
        t0 += T

    # table layouts
    idx_cols = []   # per item: per chunk (col_off, ncols, num_idx)
    msk_cols = []   # per item: col_off
    icol = mcol = 0
    for (ts, T, dc) in items:
        S = int(dc.sum())
        msk_cols.append(mcol)
        mcol += T * S
        row = []
        for c in range(NCH):
            ni = 128 * T * int(dc[c])
            row.append((icol, ni // 16, ni))
            icol += ni // 16
        idx_cols.append(row)
    IDXCOLS, MSKCOLS = icol, mcol

    # per-node lookup arrays for vectorized fill
    item_of_tile = np.zeros(NT, np.int64)
    for ii, (ts, T, dc) in enumerate(items):
        item_of_tile[ts:ts + T] = ii
    arr_ts = np.array([it[0] for it in items], np.int64)
    arr_T = np.array([it[1] for it in items], np.int64)
    arr_S = np.array([int(it[2].sum()) for it in items], np.int64)
    arr_offd = np.zeros((len(items), NCH), np.int64)
    arr_cbase = np.zeros((len(items), NCH), np.int64)
    for ii in range(len(items)):
        off = 0
        for c in range(NCH):
            arr_offd[ii, c] = off
            arr_cbase[ii, c] = idx_cols[ii][c][0]
            off += int(items[ii][2][c])
    arr_mbase = np.array(msk_cols, np.int64)

    idx_tabs, msk_tabs, vlds = [], [], []
    for c in range(NCORES):
        m = dst_core == c
        o = np.lexsort((loc_src[m], ch_src[m], dst_local[m]))
        dl = dst_local[m][o]
        ch = ch_src[m][o]
        lo = loc_src[m][o]
        ne = len(dl)
        # rank within (node, chunk)
        if ne:
            keys = dl * NCH + ch
            brk = np.ones(ne, bool)
            brk[1:] = keys[1:] != keys[:-1]
            gid = np.cumsum(brk) - 1
            gst = np.zeros(gid[-1] + 2 if ne else 1, np.int64)
            np.add.at(gst[1:], gid, 1)
            np.cumsum(gst, out=gst)
            rank = np.arange(ne) - gst[gid]
        else:
            rank = np.zeros(0, np.int64)
        til = dl // 128
        p = dl % 128
        ii = item_of_tile[til]
        t = til - arr_ts[ii]
        T = arr_T[ii]
        S = arr_S[ii]
        D = items[0][2]  # placeholder
        # gather idx table
        q = rank * (T * 128) + t * 128 + p
        col = arr_cbase[ii, ch] + q // 16
        rrow = q % 16
        it = np.zeros((16, IDXCOLS), np.int16)
        it[rrow, col] = lo.astype(np.int16)
        idx_tabs.append(np.tile(it, (8, 1)))
        # mask table
        mt = np.full((128, MSKCOLS), NEG, np.float32)
        mcolv = arr_mbase[ii] + t * S + arr_offd[ii, ch] + rank
        mt[p, mcolv] = 0.0
        # pad nodes: unmask slot (chunk0, j=0) so den=1
        vld = np.zeros((128, NT), np.float32)
        padm = order_padded[c] < 0
        for ti in range(NT):
            iii = item_of_tile[ti]
            tt = ti - arr_ts[iii]
            SS = arr_S[iii]
            prow = np.nonzero(padm[ti * 128:(ti + 1) * 128])[0]
            mt[prow, arr_mbase[iii] + tt * SS] = 0.0
            vld[:, ti] = (~padm[ti * 128:(ti + 1) * 128]).astype(np.float32)
        msk_tabs.append(mt)
        vlds.append(vld)

    return dict(
        N=N, gper=gper, NPADC=NPADC, NT=NT, NTOT=NTOT, NCH=NCH,
        order_padded=order_padded, items=items,
        idx_cols=idx_cols, msk_cols=msk_cols,
        IDXCOLS=IDXCOLS, MSKCOLS=MSKCOLS,
        idx_tabs=idx_tabs, msk_tabs=msk_tabs, vlds=vlds,
        GL=[int(v) for v in GL], GST=[int(v) for v in GST],
        roots=gstarts[:NGRAPH].copy(),
    )


# ----------------------------------------------------------------------------
# Device program
# ----------------------------------------------------------------------------

def _build_program(pp, IN, HID):
    NPADC, NT, NTOT, NCH = pp["NPADC"], pp["NT"], pp["NTOT"], pp["NCH"]
    items, idx_cols, msk_cols = pp["items"], pp["idx_cols"], pp["msk_cols"]
    IDXCOLS, MSKCOLS = pp["IDXCOLS"], pp["MSKCOLS"]
    GL, GST, gper = pp["GL"], pp["GST"], pp["gper"]
    GLMAX = int(np.ceil(max(GL) / 128) * 128)
    # actual G-buffer columns: a lone high-degree tile may exceed SLOT_BUDGET
    GC = max(T * int(dc.sum()) for (_, T, dc) in items)
    IXPAD = max(ncols for row in idx_cols for (_, ncols, _) in row)

    nc = bacc.Bacc("TRN2", target_bir_lowering=False, debug=False,
                   num_devices=(1 if NOCC else NCORES), num_swdge_queues=4)

    xT = nc.dram_tensor("xT", [IN, NPADC], F32, kind="ExternalInput")
    xrootT = nc.dram_tensor("xrootT", [IN, gper], F32, kind="ExternalInput")
    idx_t = nc.dram_tensor("idx", [128, IDXCOLS], I16, kind="ExternalInput")
    msk_t = nc.dram_tensor("msk", [128, MSKCOLS], F32, kind="ExternalInput")
    vld_t = nc.dram_tensor("vld", [128, NT], F32, kind="ExternalInput")
    Ws = {}
    for l, di in ((1, IN), (2, HID), (3, HID)):
        Ws[f"W{l}"] = nc.dram_tensor(f"W{l}", [di, HID], F32, kind="ExternalInput")
        Ws[f"as{l}"] = nc.dram_tensor(f"as{l}", [HID, 1], F32, kind="ExternalInput")
        Ws[f"ad{l}"] = nc.dram_tensor(f"ad{l}", [HID, 1], F32, kind="ExternalInput")
        Ws[f"b{l}"] = nc.dram_tensor(f"b{l}", [128, HID], F32, kind="ExternalInput")
    lin0W = nc.dram_tensor("lin0W", [HID, HID], F32, kind="ExternalInput")
    lin0b = nc.dram_tensor("lin0b", [gper, HID], F32, kind="ExternalInput")
    linnW = nc.dram_tensor("linnW", [IN, HID], F32, kind="ExternalInput")
    linnb = nc.dram_tensor("linnb", [gper, HID], F32, kind="ExternalInput")
    lin1W = nc.dram_tensor("lin1W", [2 * HID, 1], F32, kind="ExternalInput")
    lin1b = nc.dram_tensor("lin1b", [gper, 1], F32, kind="ExternalInput")
    ident_in = nc.dram_tensor("ident", [128, 128], F32, kind="ExternalInput")
    out_t = nc.dram_tensor("out", [gper, 1], F32, kind="ExternalOutput")

    agin = [nc.dram_tensor(f"agin{l}", [NPADC, ROW], BF16, kind="Internal")
            for l in range(3)]
    htab = [nc.dram_tensor(f"htab{l}", [NTOT, ROW], BF16, kind="Internal",
                           addr_space="Shared")
            for l in range(3)]
    x4T_d = nc.dram_tensor("x4T", [HID, NPADC], F32, kind="Internal")

    with tile.TileContext(nc) as tc:
        with (
            tc.tile_pool(name="const", bufs=1) as cpool,
            tc.tile_pool(name="gbuf", bufs=2) as gpool,
            tc.tile_pool(name="pbuf", bufs=2) as ppool,
            tc.tile_pool(name="sbuf", bufs=3) as spool,
            tc.tile_pool(name="psum", bufs=2, space="PSUM") as pspool,
            tc.tile_pool(name="psA", bufs=2, space="PSUM") as psA,
        ):
            ident = cpool.tile([128, 128], F32, tag="ident")
            nc.sync.dma_start(ident[:], ident_in[:])

            # Wcat_l = [W_l | W_l@a_src | W_l@a_dst], plus bias broadcast
            wcat = []
            s_dst_res = []
            for l, di in ((1, IN), (2, HID), (3, HID)):
                w_sb = cpool.tile([di, HID], F32, tag=f"w{l}")
                nc.sync.dma_start(w_sb[:], Ws[f"W{l}"][:])
                wc = cpool.tile([di, HID + 2], F32, tag=f"wc{l}")
                nc.vector.tensor_copy(wc[:, :HID], w_sb[:])
                if NOPRO:
                    nc.vector.memset(wc[:, HID:], 0.01)
                else:
                    ps_wt = psA.tile([HID, 128], F32, space="PSUM", tag="aux", name="ps_wt")
                    nc.tensor.transpose(ps_wt[:, :di], w_sb[:], ident[:di, :di])
                    wt_sb = cpool.tile([HID, 128], F32, tag=f"wt{l}")
                    nc.scalar.copy(wt_sb[:, :di], ps_wt[:, :di])
                    for name, col in ((f"as{l}", HID), (f"ad{l}", HID + 1)):
                        a_sb = cpool.tile([HID, 1], F32, tag=f"t{name}")
                        nc.sync.dma_start(a_sb[:], Ws[name][:])
                        ps_wa = psA.tile([128, 1], F32, space="PSUM", tag="aux", name="ps_wa")
                        nc.tensor.matmul(ps_wa[:di, :], wt_sb[:, :di], a_sb[:])
                        nc.vector.tensor_copy(wc[:, col:col + 1], ps_wa[:di, :])
                wcat.append(wc)
                b_sb = cpool.tile([128, HID], F32, tag=f"bb{l}")
                nc.sync.dma_start(b_sb[:], Ws[f"b{l}"][:])
                Ws[f"bsb{l}"] = b_sb
                s_dst_res.append(cpool.tile([128, NT], F32, tag=f"sdst{l}", name=f"sdst{l}"))

            vld_sb = cpool.tile([128, NT], F32, tag="vld")
            nc.sync.dma_start(vld_sb[:], vld_t[:])

            # phase A, layer 1
            for t in range(NT):
                x_sb = spool.tile([IN, 128], F32, tag="ax")
                nc.sync.dma_start(x_sb[:], xT[:, t * 128:(t + 1) * 128])
                ps_h = psA.tile([128, HID + 2], F32, space="PSUM", tag="ph", name="ps_h")
                nc.tensor.matmul(ps_h[:], x_sb[:], wcat[0][:])
                hx = spool.tile([128, ROW], BF16, tag="hx")
                nc.vector.memset(hx[:, HID + 1:], 0.0)
                nc.scalar.copy(hx[:, :HID + 1], ps_h[:, :HID + 1])
                nc.vector.tensor_copy(s_dst_res[0][:, t:t + 1],
                                      ps_h[:, HID + 1:HID + 2])
                nc.sync.dma_start(agin[0][t * 128:(t + 1) * 128, :], hx[:])

            # 3 GAT layers
            nlayers = 3 if DBG == 0 else 1
            for l in range(nlayers):
                if NOCC:
                    nc.sync.dma_start(htab[l][0:NPADC, :], agin[l][:])
                else:
                    nc.gpsimd.collective_compute(
                        "AllGather", ALU.bypass,
                        replica_groups=[list(range(NCORES))],
                        ins=[agin[l][:]], outs=[htab[l][:]],
                    )
                for ii, (ts, T, dc) in enumerate(items):
                    if DBG == 1 or ii >= MAXITEMS:
                        break
                    S = int(dc.sum())
                    gq = 0
                    G_sb = gpool.tile([128, GC, ROW], BF16, tag="G")
                    goff = G_sb[:].offset
                    offd = 0
                    for chn in range(min(NCH, MAXCH)):
                        D = int(dc[chn])
                        cbase, ncols, ni = idx_cols[ii][chn]
                        rows_c = min(CHUNK, NTOT - chn * CHUNK)
                        ix = spool.tile([128, ncols], I16, tag="ix",
                                        padded_shape=[128, IXPAD])
                        nc.sync.dma_start(ix[:],
                                          idx_t[:, cbase:cbase + ncols])
                        in_ap = _ap(htab[l], chn * CHUNK * ROW,
                                    [(ROW, rows_c), (1, ROW)])
                        # split ni into even 128-multiple calls <= GMAX so
                        # each call fits one <=33-desc/lane packet
                        ncalls = max(1, -(-ni // GMAX))
                        csz = -(-(ni // 128) // ncalls) * 128
                        off = 0
                        while off < ni:
                            sni = min(csz, ni - off)
                            out_ap = _ap(
                                G_sb.tensor,
                                goff + (offd * T + off // 128) * ROW,
                                [(GC * ROW, 128), (ROW, sni // 128), (1, ROW)])
                            nc.gpsimd.dma_gather(
                                out_ap, in_ap,
                                ix[:, off // 16:(off + sni) // 16],
                                sni, sni, ROW, single_packet=True,
                                queue_num=gq % 4)
                            gq += 1
                            off += sni
                        offd += D
                    if DBG == 2:
                        continue
                    mbase = msk_cols[ii]
                    mk = spool.tile([128, 128], F32, tag="mk")
                    nc.sync.dma_start(mk[:, :T * S],
                                      msk_t[:, mbase:mbase + T * S])
                    mtv = _ap(mk.tensor, mk[:].offset,
                              [(128, 128), (S, T), (1, S)])
                    ssv = _ap(G_sb.tensor, goff + HID,
                              [(GC * ROW, 128), (T * ROW, S), (ROW, T)])
                    e_sb = spool.tile([128, 128], F32, tag="e")
                    ev = _ap(e_sb.tensor, e_sb[:].offset,
                             [(128, 128), (1, S), (S, T)])
                    nc.vector.tensor_copy(ev, ssv)
                    et = _ap(e_sb.tensor, e_sb[:].offset,
                             [(128, 128), (S, T), (1, S)])
                    nc.vector.tensor_tensor(et, et, mtv, ALU.add)
                    sdv = _ap(s_dst_res[l].tensor, s_dst_res[l][:].offset + ts,
                              [(NT, 128), (1, T), (0, S)])
                    nc.vector.tensor_tensor(et, et, sdv, ALU.add)
                    e2_sb = spool.tile([128, 128], F32, tag="e2")
                    e2t = _ap(e2_sb.tensor, e2_sb[:].offset,
                              [(128, 128), (S, T), (1, S)])
                    nc.scalar.activation(e2t, et, ACTF.Copy, scale=0.2)
                    nc.vector.tensor_tensor(et, et, e2t, ALU.max)
                    red = spool.tile([128, MAX_TILES, 4], F32, tag="red")
                    nc.vector.tensor_reduce(red[:, :T, 0:1], et, AX.X, ALU.max)
                    mxb = _ap(red.tensor, red[:].offset,
                              [(MAX_TILES * 4, 128), (4, T), (0, S)])
                    nc.vector.tensor_tensor(et, et, mxb, ALU.subtract)
                    nc.scalar.activation(et, et, ACTF.Exp)
                    nc.vector.tensor_reduce(red[:, :T, 1:2], et, AX.X, ALU.add)
                    nc.vector.reciprocal(red[:, :T, 2:3], red[:, :T, 1:2])
                    nb = spool.tile([128, 128], BF16, tag="nb")
                    nbt = _ap(nb.tensor, nb[:].offset,
                              [(128, 128), (S, T), (1, S)])
                    nc.vector.tensor_copy(nbt, et)
                    # P[t][j][f] = G_h * num
                    P_sb = ppool.tile([128, GC, HID], BF16, tag="P")
                    poff = P_sb[:].offset
                    ghv = _ap(G_sb.tensor, goff,
                              [(GC * ROW, 128), (T * ROW, S), (ROW, T), (1, HID)])
                    nbv = _ap(nb.tensor, nb[:].offset,
                              [(128, 128), (1, S), (S, T), (0, HID)])
                    pv = _ap(P_sb.tensor, poff,
                             [(GC * HID, 128), (HID, S), (S * HID, T), (1, HID)])
                    nc.any.tensor_tensor(pv, ghv, nbv, ALU.mult)
                    o_sb = spool.tile([128, MAX_TILES, HID], F32, tag="o")
                    prd = _ap(P_sb.tensor, poff,
                              [(GC * HID, 128), (S * HID, T), (1, HID), (HID, S)])
                    nc.vector.tensor_reduce(o_sb[:, :T, :], prd, AX.X, ALU.add)
                    rdb = _ap(red.tensor, red[:].offset + 2,
                              [(MAX_TILES * 4, 128), (4, T), (0, HID)])
                    nc.vector.tensor_tensor(o_sb[:, :T, :], o_sb[:, :T, :],
                                            rdb, ALU.mult)
                    bb = _ap(Ws[f"bsb{l + 1}" if l < 2 else "bsb3"].tensor,
                             Ws[f"bsb{l + 1}" if l < 2 else "bsb3"][:].offset,
                             [(HID, 128), (0, T), (1, HID)])
                    nc.vector.tensor_tensor(o_sb[:, :T, :], o_sb[:, :T, :],
                                            bb, ALU.add)
                    nc.scalar.activation(o_sb[:, :T, :], o_sb[:, :T, :],
                                         ACTF.Relu)
                    if l == 2:
                        vb = _ap(vld_sb.tensor, vld_sb[:].offset + ts,
                                 [(NT, 128), (1, T), (0, HID)])
                        nc.vector.tensor_tensor(o_sb[:, :T, :], o_sb[:, :T, :],
                                                vb, ALU.mult)
                    if DBG == 3:
                        continue
                    for t in range(T):
                        ps_t = pspool.tile([HID, 128], F32, space="PSUM")
                        nc.tensor.transpose(ps_t[:], o_sb[:, t, :], ident[:])
                        xt_sb = spool.tile([HID, 128], F32, tag="xt")
                        nc.scalar.copy(xt_sb[:], ps_t[:])
                        if l < 2:
                            ps_h = psA.tile([128, HID + 2], F32, space="PSUM", tag="ph", name="ps_h")
                            nc.tensor.matmul(ps_h[:], xt_sb[:], wcat[l + 1][:])
                            hx = spool.tile([128, ROW], BF16, tag="hx")
                            nc.vector.memset(hx[:, HID + 1:], 0.0)
                            nc.scalar.copy(hx[:, :HID + 1], ps_h[:, :HID + 1])
                            nc.vector.tensor_copy(
                                s_dst_res[l + 1][:, ts + t:ts + t + 1],
                                ps_h[:, HID + 1:HID + 2])
                            nc.sync.dma_start(
                                agin[l + 1][(ts + t) * 128:(ts + t + 1) * 128, :],
                                hx[:])
                        else:
                            nc.sync.dma_start(
                                x4T_d[:, (ts + t) * 128:(ts + t + 1) * 128],
                                xt_sb[:])

            # head
            if DBG:
                o_dbg = cpool.tile([gper, 1], F32, tag="odbg")
                nc.vector.memset(o_dbg[:], 0.5)
                nc.sync.dma_start(out_t[:], o_dbg[:])
            hmaxT = cpool.tile([HID, gper], F32, tag="hmaxT")
            if DBG:
                hmaxT = None
            for g in range(gper if not DBG else 0):
                hg = spool.tile([HID, GLMAX], F32, tag="hg")
                nc.sync.dma_start(hg[:, :GL[g]], x4T_d[:, GST[g]:GST[g] + GL[g]])
                nc.vector.tensor_reduce(hmaxT[:, g:g + 1], hg[:, :GL[g]],
                                        AX.X, ALU.max)
            if not DBG:
                lw_sb = cpool.tile([HID, HID], F32, tag="l0w")
                nc.sync.dma_start(lw_sb[:], lin0W[:])
                ps_g = psA.tile([gper, HID], F32, space="PSUM", tag="aux", name="ps_g")
                nc.tensor.matmul(ps_g[:], hmaxT[:], lw_sb[:])
                b0_sb = cpool.tile([gper, HID], F32, tag="l0b")
                nc.sync.dma_start(b0_sb[:], lin0b[:])
                h0 = cpool.tile([gper, HID], F32, tag="h0")
                nc.vector.tensor_tensor(h0[:], ps_g[:], b0_sb[:], ALU.add)
                nc.scalar.activation(h0[:], h0[:], ACTF.Relu)

                xr_sb = cpool.tile([IN, gper], F32, tag="xr")
                nc.sync.dma_start(xr_sb[:], xrootT[:])
                lnw_sb = cpool.tile([IN, HID], F32, tag="lnw")
                nc.sync.dma_start(lnw_sb[:], linnW[:])
                ps_n = psA.tile([gper, HID], F32, space="PSUM", tag="aux", name="ps_n")
                nc.tensor.matmul(ps_n[:], xr_sb[:], lnw_sb[:])
                bn_sb = cpool.tile([gper, HID], F32, tag="lnb")
                nc.sync.dma_start(bn_sb[:], linnb[:])
                hn = cpool.tile([gper, HID], F32, tag="hn")
                nc.vector.tensor_tensor(hn[:], ps_n[:], bn_sb[:], ALU.add)
                nc.scalar.activation(hn[:], hn[:], ACTF.Relu)

                catT = cpool.tile([2 * HID, gper], F32, tag="catT")
                ps_t0 = psA.tile([HID, gper], F32, space="PSUM", tag="aux", name="ps_t0")
                nc.tensor.transpose(ps_t0[:], h0[:], ident[:gper, :gper])
                nc.scalar.copy(catT[:HID, :], ps_t0[:])
                ps_t1 = psA.tile([HID, gper], F32, space="PSUM", tag="aux", name="ps_t1")
                nc.tensor.transpose(ps_t1[:], hn[:], ident[:gper, :gper])
                nc.scalar.copy(catT[HID:, :], ps_t1[:])

                l1w_sb = cpool.tile([2 * HID, 1], F32, tag="l1w")
                nc.sync.dma_start(l1w_sb[:], lin1W[:])
                ps_o = psA.tile([gper, 1], F32, space="PSUM", tag="aux", name="ps_o")
                nc.tensor.matmul(ps_o[:], catT[:], l1w_sb[:])
                b1_sb = cpool.tile([gper, 1], F32, tag="l1b")
                nc.sync.dma_start(b1_sb[:], lin1b[:])
                o_fin = cpool.tile([gper, 1], F32, tag="ofin")
                nc.scalar.activation(o_fin[:], ps_o[:], ACTF.Sigmoid, bias=b1_sb[:])
                nc.sync.dma_start(out_t[:], o_fin[:])

    nc.compile()
    return nc


# ----------------------------------------------------------------------------
# entry point
# ----------------------------------------------------------------------------

_CACHE = {}
LAST_RESULTS = None
LAST_NC = None
LAST_INMAPS = None


def kernel(x, adj, batch, W1, a_src1, a_dst1, b1, W2, a_src2, a_dst2, b2,
           W3, a_src3, a_dst3, b3, linnews_W, linnews_b, lin0_W, lin0_b,
           lin1_W, lin1_b):
    x = np.asarray(x)
    adj = np.asarray(adj)
    batch = np.asarray(batch)
    N, IN = x.shape
    HID = np.asarray(W1).shape[1]
    gper = NGRAPH // NCORES

    ckey = (N, adj.shape[1], IN, HID,
            hash(adj.tobytes()), hash(batch.tobytes()))
    if ckey in _CACHE:
        pp, nc = _CACHE[ckey]
    else:
        pp = _preprocess(adj, batch)
        nc = _build_program(pp, IN, HID)
        _CACHE.clear()
        _CACHE[ckey] = (pp, nc)

    NPADC = pp["NPADC"]
    order_padded = pp["order_padded"]
    f32 = np.float32
    in_maps = []
    for c in range(NCORES):
        oc = order_padded[c]
        xc = np.zeros((NPADC, IN), f32)
        real = oc >= 0
        xc[real] = np.asarray(x, f32)[oc[real]]
        roots = pp["roots"][c * gper:(c + 1) * gper]
        im = {
            "xT": np.ascontiguousarray(xc.T),
            "xrootT": np.ascontiguousarray(np.asarray(x, f32)[roots].T),
            "idx": pp["idx_tabs"][c],
            "msk": pp["msk_tabs"][c],
            "vld": pp["vlds"][c],
            "W1": np.asarray(W1, f32), "W2": np.asarray(W2, f32),
            "W3": np.asarray(W3, f32),
            "as1": np.asarray(a_src1, f32).reshape(HID, 1),
            "ad1": np.asarray(a_dst1, f32).reshape(HID, 1),
            "as2": np.asarray(a_src2, f32).reshape(HID, 1),
            "ad2": np.asarray(a_dst2, f32).reshape(HID, 1),
            "as3": np.asarray(a_src3, f32).reshape(HID, 1),
            "ad3": np.asarray(a_dst3, f32).reshape(HID, 1),
            "b1": np.tile(np.asarray(b1, f32).reshape(1, HID), (128, 1)),
            "b2": np.tile(np.asarray(b2, f32).reshape(1, HID), (128, 1)),
            "b3": np.tile(np.asarray(b3, f32).reshape(1, HID), (128, 1)),
            "lin0W": np.asarray(lin0_W, f32),
            "lin0b": np.tile(np.asarray(lin0_b, f32).reshape(1, HID), (gper, 1)),
            "linnW": np.asarray(linnews_W, f32),
            "linnb": np.tile(np.asarray(linnews_b, f32).reshape(1, HID),
                             (gper, 1)),
            "lin1W": np.asarray(lin1_W, f32).reshape(2 * HID, 1),
            "lin1b": np.tile(np.asarray(lin1_b, f32).reshape(1, 1), (gper, 1)),
            "ident": np.eye(128, dtype=f32),
        }
        in_maps.append(im)

    kw = {}
    if os.environ.get("GAT_TRACE", "0") == "1":
        kw = dict(trace=True)
    global LAST_RESULTS, LAST_NC, LAST_INMAPS
    LAST_NC, LAST_INMAPS = nc, in_maps
    res = run_bass_kernel_spmd(nc, in_maps, core_ids=list(range(NCORES)), **kw)
    LAST_RESULTS = res
    out = np.concatenate([res.results[c]["out"] for c in range(NCORES)], axis=0)
    return out.astype(np.float32)

